# revision 2
# baseline (speedup 1.0000x reference)
"""nn_NeuralODE TRN2 kernel: 100 SSP-RK3 steps of a learned 16-channel
1D stencil (conv k=5) with ghost-cell BCs on z (32,16,8192) fp32.

Sharding: data-parallel over batch (4 per core x 8 NeuronCores); each core
splits its 8192-col spatial domain into 2 halves across partitions ->
128 partitions = (half, batch, channel), free dim = columns.

Algorithm: the RK3 step is linear in z, so one step is z' = (I + K1) z with
K1 = h*C + h^2/2 C^2 + h^3/6 C^3 (C = the 5-tap stencil operator; K1 radius 6).
Twenty steps collapse into ONE convolution: K20 = (I+K1)^20 - I, whose tap
coefficients decay factorially -> truncated to radius 10 (21 taps, dropped
norm ~3e-11). The main track applies K20 as 21 accumulating fp32r matmuls
(block-diagonal 128x128 weights, 8 groups of 16 channels) per 512-col PSUM
tile + one vector add per tile, 5 times total.

Ghost-cell boundary effects invalidate 6n=120 cols per global edge per
application; those are recomputed exactly by 20 stepwise RK3 steps on a
[128, 264] gathered strip window (left edge in partitions 0..63, right edge
in 64..127, sharing instructions). Ghost cols 8,9 / L-10,L-9 are maintained
on device (the only ghost cells interior reads ever touch); the full 10-col
ghost regions are filled host-side at the end.
"""
import numpy as np

import concourse.bacc as bacc
import concourse.mybir as mybir
from concourse.tile import TileContext
from concourse.bass_utils import run_bass_kernel_spmd

F32 = mybir.dt.float32
F32R = mybir.dt.float32r
IGST = 10
ALU = mybir.AluOpType
RADN = 10              # n-step kernel truncation radius
NKT = 2 * RADN + 1     # 21 taps
EPS = 0.01
NCORES = 8


def _round_fp32r(x):
    b = np.ascontiguousarray(x, dtype=np.float32).view(np.uint32)
    return (b & np.uint32(0xFFFFF000)).view(np.float32).copy()


def _ksp(Ka, Kb, rad):
    out = {}
    for da, Ma in Ka.items():
        for db, Mb in Kb.items():
            d = da + db
            if abs(d) <= rad:
                out[d] = out.get(d, 0) + Ma @ Mb
    return out


def _nstep_K(W, h, n, radius):
    C = {k - 2: W[:, :, k].astype(np.float64) for k in range(5)}
    C2 = _ksp(C, C, 99)
    C3 = _ksp(C2, C, 99)
    K1 = {}
    for d in set(C) | set(C2) | set(C3):
        K1[d] = h * C.get(d, 0) + h * h / 2 * C2.get(d, 0) + h ** 3 / 6 * C3.get(d, 0)
    I16 = np.eye(16)
    P = {d: (M + (I16 if d == 0 else 0)) for d, M in K1.items()}
    result = {0: I16.copy()}
    base = P
    e = n
    while e:
        if e & 1:
            result = _ksp(result, base, radius + 4)
        e >>= 1
        if e:
            base = _ksp(base, base, radius + 4)
    return {d: (M - (I16 if d == 0 else 0)) for d, M in result.items()
            if abs(d) <= radius}


def _make_weights(W, h, n):
    Kn = _nstep_K(W, h, n, RADN)
    mats = [Kn.get(j - RADN, np.zeros((16, 16))).T for j in range(NKT)]
    for coef in (h, 4.0 * h):
        for k in range(5):
            mats.append(coef * W[:, :, k].astype(np.float64).T)
    out = np.zeros((len(mats), 128, 128), np.float64)
    for i, blk in enumerate(mats):
        for g in range(8):
            out[i, g * 16:(g + 1) * 16, g * 16:(g + 1) * 16] = blk
    return _round_fp32r(out.reshape(len(mats) * 128, 128).astype(np.float32))


def _pack_z(z, R=RADN):
    B, nch, L = z.shape
    I = L // 2
    C = I + 2 * R
    rows = np.ascontiguousarray(
        z.reshape(B, 16, 2, I).transpose(2, 0, 1, 3).reshape(128, I), dtype=np.float32)
    buf = np.empty((128, C), np.float32)
    buf[:, R:R + I] = rows
    buf[0:64, 0:R] = rows[0:64, 0:R]
    buf[64:128, 0:R] = rows[0:64, I - R:I]
    buf[0:64, R + I:C] = rows[64:128, 0:R]
    buf[64:128, R + I:C] = rows[64:128, I - R:I]
    return buf


def _unpack_z(buf, B, L):
    I = L // 2
    z = buf.reshape(2, B, 16, I).transpose(1, 2, 0, 3).reshape(B, 16, L).copy()
    z[:, :, :IGST] = z[:, :, IGST:IGST + 1]
    z[:, :, L - IGST:] = z[:, :, L - IGST - 1:L - IGST]
    return z


def _build_nc(n_steps, I, TW, nfuse):
    assert I % TW == 0 and n_steps % nfuse == 0
    NT = I // TW
    R = RADN
    C = I + 2 * R
    n = nfuse
    Wz = 12 * n + 24
    WA = Wz - 4
    napp = n_steps // n
    repl = 6 * n
    nc = bacc.Bacc(None, target_bir_lowering=False)

    zin = nc.declare_dram_parameter("zin", [128, C], F32, isOutput=False)
    win = nc.declare_dram_parameter("wbd", [(NKT + 10) * 128, 128], F32R, isOutput=False)
    zout_d = nc.declare_dram_parameter("zout", [128, I], F32, isOutput=True)

    with TileContext(nc) as tc:
        with tc.tile_pool(name="state", bufs=1) as st, \
             tc.tile_pool(name="wpool", bufs=1) as wp, \
             tc.tile_pool(name="psA", bufs=5, space="PSUM") as psA, \
             tc.tile_pool(name="psS", bufs=3, space="PSUM") as psS:
            za = st.tile([128, C], F32, tag="zA", name="za")
            zb = st.tile([128, C], F32, tag="zB", name="zb")
            zbufs = [za, zb]
            zr = st.tile([128, C], F32R, tag="zr", name="zr")
            zsA = st.tile([128, Wz], F32, tag="zsA", name="zsA")
            zsB = st.tile([128, Wz], F32, tag="zsB", name="zsB")
            zsbufs = [zsA, zsB]
            zsr = st.tile([128, Wz], F32R, tag="zsr", name="zsr")
            k1s = st.tile([128, Wz], F32R, tag="k1s", name="k1s")
            k2s = st.tile([128, Wz], F32R, tag="k2s", name="k2s")
            wf, wm = [], {}
            for j in range(NKT):
                w = wp.tile([128, 128], F32R, tag=f"wf{j}", name=f"wf{j}")
                nc.sync.dma_start(out=w[:, :], in_=win[j * 128:(j + 1) * 128, :])
                wf.append(w)
            for v in range(2):
                for k in range(5):
                    idx = NKT + v * 5 + k
                    w = wp.tile([128, 128], F32R, tag=f"wm{idx}", name=f"wm{idx}")
                    nc.sync.dma_start(out=w[:, :], in_=win[idx * 128:(idx + 1) * 128, :])
                    wm[(v, k)] = w

            nc.vector.memset(k1s[:, :].bitcast(F32), 0.0)
            nc.vector.memset(k2s[:, :].bitcast(F32), 0.0)
            nc.sync.dma_start(out=za[:, :], in_=zin[:, :])
            for t in range(NT):
                s = R + TW * t
                nc.scalar.copy(zr[:, s:s + TW], za[:, s:s + TW])
            nc.scalar.copy(zr[:, 0:R], za[:, 0:R])
            nc.scalar.copy(zr[:, R + I:C], za[:, R + I:C])

            for app in range(napp):
                z = zbufs[app % 2]
                znew = zbufs[(app + 1) % 2]
                last = app == napp - 1

                nc.scalar.copy(zsA[0:64, :], z[0:64, R + 8:R + 8 + Wz])
                nc.scalar.copy(zsA[64:128, :], z[64:128, I + R - 8 - Wz:I + R - 8])
                nc.vector.tensor_copy(zsr[0:64, :], z[0:64, R + 8:R + 8 + Wz])
                nc.vector.tensor_copy(zsr[64:128, :], z[64:128, I + R - 8 - Wz:I + R - 8])

                def main_tile(t):
                    a = psA.tile([128, TW], F32, tag="A", name=f"A{app}_{t}")
                    for j in range(NKT):
                        nc.tensor.matmul(a[:, :], wf[j][:, :],
                                         zr[:, TW * t + j: TW * t + j + TW],
                                         start=(j == 0), stop=(j == NKT - 1),
                                         skip_group_check=True)
                    s = R + TW * t
                    nc.vector.tensor_tensor(znew[:, s:s + TW], a[:, :], z[:, s:s + TW],
                                            ALU.add)

                main_at = {}
                for idx in range(NT):
                    main_at.setdefault(min(2 * idx, n - 1), []).append(idx)

                for k_step in range(n):
                    zs = zsbufs[k_step % 2]
                    zs_n = zsbufs[(k_step + 1) % 2]
                    As = psS.tile([128, WA], F32, tag="As", name=f"As{app}_{k_step}")

                    def sstage(v, rhs, first=False):
                        for k in range(5):
                            nc.tensor.matmul(As[:, :], wm[(v, k)][:, :],
                                             rhs[:, k:k + WA],
                                             start=(first and k == 0),
                                             stop=(v == 1 and k == 4),
                                             skip_group_check=True)

                    def sfix(buf):
                        nc.vector.tensor_copy(
                            buf[0:64, 0:2], buf[0:64, 2:3].broadcast_to([64, 2]))
                        nc.vector.tensor_copy(
                            buf[64:128, Wz - 2:Wz],
                            buf[64:128, Wz - 3:Wz - 2].broadcast_to([64, 2]))

                    sstage(0, zsr, first=True)
                    nc.vector.tensor_tensor(k1s[:, 2:2 + WA], As[:, :],
                                            zs[:, 2:2 + WA], ALU.add)
                    sfix(k1s)
                    sstage(0, k1s)
                    nc.vector.scalar_tensor_tensor(k2s[:, 2:2 + WA], As[:, :], 0.25,
                                                   zs[:, 2:2 + WA], ALU.mult, ALU.add)
                    sfix(k2s)
                    sstage(1, k2s)
                    nc.vector.scalar_tensor_tensor(zs_n[:, 2:2 + WA], As[:, :],
                                                   1.0 / 6.0, zs[:, 2:2 + WA],
                                                   ALU.mult, ALU.add)
                    sfix(zs_n)
                    if k_step < n - 1:
                        nc.vector.tensor_copy(zsr[:, :], zs_n[:, :])
                    for t in main_at.get(k_step, []):
                        main_tile(t)

                zs_fin = zsbufs[n % 2]
                nc.vector.tensor_copy(znew[0:64, R + 10:R + 10 + repl],
                                      zs_fin[0:64, 2:2 + repl])
                nc.vector.tensor_copy(znew[64:128, I - repl:I],
                                      zs_fin[64:128, Wz - 2 - repl:Wz - 2])

                if not last:
                    nc.scalar.copy(znew[0:64, R + 8:R + 10],
                                   znew[0:64, R + 10:R + 11].broadcast_to([64, 2]))
                    nc.scalar.copy(znew[64:128, I:I + 2],
                                   znew[64:128, I - 1:I].broadcast_to([64, 2]))
                    nc.sync.dma_start(out=znew[0:64, R + I:C], in_=znew[64:128, R:2 * R])
                    nc.sync.dma_start(out=znew[64:128, 0:R], in_=znew[0:64, I:I + R])
                    for t in range(NT):
                        s = R + TW * t
                        nc.scalar.copy(zr[:, s:s + TW], znew[:, s:s + TW])
                    nc.scalar.copy(zr[:, 0:R], znew[:, 0:R])
                    nc.scalar.copy(zr[:, R + I:C], znew[:, R + I:C])

            zfin = zbufs[napp % 2]
            nc.sync.dma_start(out=zout_d[:, :], in_=zfin[:, R:R + I])

    nc.finalize()
    return nc


_NC_CACHE = {}


def _pick_nfuse(n_steps):
    for n in (20, 10, 5, 4, 2, 1):
        if n_steps % n == 0 and 6 * n < 1500:
            return n
    return 1


def kernel(z0, W, t1_t0):
    z0 = np.asarray(z0, dtype=np.float32)
    W = np.asarray(W, dtype=np.float32)
    t = int(np.asarray(t1_t0))
    if t == 0:
        return z0.copy()
    n_steps = int(round(t / EPS))
    h = t / n_steps
    B, nch, L = z0.shape
    I = L // 2
    BPC = B // NCORES
    nfuse = _pick_nfuse(n_steps)

    key = (n_steps, I, nfuse)
    if key not in _NC_CACHE:
        _NC_CACHE[key] = _build_nc(n_steps, I, 512, nfuse)
    nc = _NC_CACHE[key]

    wbd = _make_weights(W, h, nfuse)
    in_maps = [{"zin": _pack_z(z0[c * BPC:(c + 1) * BPC]), "wbd": wbd}
               for c in range(NCORES)]
    br = run_bass_kernel_spmd(nc, in_maps, list(range(NCORES)))
    out = np.concatenate(
        [_unpack_z(br.results[c]["zout"], BPC, L) for c in range(NCORES)], axis=0)
    return out.astype(np.float32)


# revision 3
# speedup vs baseline: 1.1583x; 1.1583x over previous
"""nn_NeuralODE TRN2 kernel: 100 SSP-RK3 steps of a learned 16-channel
1D stencil (conv k=5) with ghost-cell BCs on z (32,16,8192) fp32.

Sharding: data-parallel over batch (4 per core x 8 NeuronCores); each core
splits its 8192-col spatial domain into 2 halves across partitions ->
128 partitions = (half, batch, channel), free dim = columns.

Algorithm: the RK3 step is linear in z, so one step is z' = (I + K1) z with
K1 = h*C + h^2/2 C^2 + h^3/6 C^3 (C = the 5-tap stencil operator; K1 radius 6).
Twenty steps collapse into ONE convolution: K20 = (I+K1)^20 - I, whose tap
coefficients decay factorially -> truncated to radius 10 (21 taps, dropped
norm ~3e-11). The main track applies K20 as 21 accumulating fp32r matmuls
(block-diagonal 128x128 weights, 8 groups of 16 channels) per 512-col PSUM
tile + one vector add per tile, 5 times total.

Ghost-cell boundary effects invalidate 6n=120 cols per global edge per
application; those are recomputed exactly by 20 stepwise RK3 steps on a
[128, 264] gathered strip window (left edge in partitions 0..63, right edge
in 64..127, sharing instructions). Ghost cols 8,9 / L-10,L-9 are maintained
on device (the only ghost cells interior reads ever touch); the full 10-col
ghost regions are filled host-side at the end.
"""
import numpy as np

import concourse.bacc as bacc
import concourse.mybir as mybir
from concourse.tile import TileContext
from concourse.bass_utils import run_bass_kernel_spmd

F32 = mybir.dt.float32
F32R = mybir.dt.float32r
BF16 = mybir.dt.bfloat16
IGST = 10
ALU = mybir.AluOpType
RADN = 10              # n-step kernel truncation radius
NKT = 2 * RADN + 1     # 21 taps
EPS = 0.01
NCORES = 8


def _round_fp32r(x):
    b = np.ascontiguousarray(x, dtype=np.float32).view(np.uint32)
    return (b & np.uint32(0xFFFFF000)).view(np.float32).copy()


def _ksp(Ka, Kb, rad):
    out = {}
    for da, Ma in Ka.items():
        for db, Mb in Kb.items():
            d = da + db
            if abs(d) <= rad:
                out[d] = out.get(d, 0) + Ma @ Mb
    return out


def _nstep_K(W, h, n, radius):
    C = {k - 2: W[:, :, k].astype(np.float64) for k in range(5)}
    C2 = _ksp(C, C, 99)
    C3 = _ksp(C2, C, 99)
    K1 = {}
    for d in set(C) | set(C2) | set(C3):
        K1[d] = h * C.get(d, 0) + h * h / 2 * C2.get(d, 0) + h ** 3 / 6 * C3.get(d, 0)
    I16 = np.eye(16)
    P = {d: (M + (I16 if d == 0 else 0)) for d, M in K1.items()}
    result = {0: I16.copy()}
    base = P
    e = n
    while e:
        if e & 1:
            result = _ksp(result, base, radius + 4)
        e >>= 1
        if e:
            base = _ksp(base, base, radius + 4)
    return {d: (M - (I16 if d == 0 else 0)) for d, M in result.items()
            if abs(d) <= radius}


def _bd_stack(mats):
    out = np.zeros((len(mats), 128, 128), np.float64)
    for i, blk in enumerate(mats):
        for g in range(8):
            out[i, g * 16:(g + 1) * 16, g * 16:(g + 1) * 16] = blk
    return out.reshape(len(mats) * 128, 128)


def _make_weights(W, h, n):
    import ml_dtypes
    Kn = _nstep_K(W, h, n, RADN)
    mats = [Kn.get(j - RADN, np.zeros((16, 16))).T for j in range(NKT)]
    wbd = _round_fp32r(_bd_stack(mats).astype(np.float32))
    mini = []
    for coef in (h, 4.0 * h):
        for k in range(5):
            mini.append(coef * W[:, :, k].astype(np.float64).T)
    wmb = _bd_stack(mini).astype(np.float32).astype(ml_dtypes.bfloat16)
    return wbd, wmb


def _pack_z(z, R=RADN):
    B, nch, L = z.shape
    I = L // 2
    C = I + 2 * R
    rows = np.ascontiguousarray(
        z.reshape(B, 16, 2, I).transpose(2, 0, 1, 3).reshape(128, I), dtype=np.float32)
    buf = np.empty((128, C), np.float32)
    buf[:, R:R + I] = rows
    buf[0:64, 0:R] = rows[0:64, 0:R]
    buf[64:128, 0:R] = rows[0:64, I - R:I]
    buf[0:64, R + I:C] = rows[64:128, 0:R]
    buf[64:128, R + I:C] = rows[64:128, I - R:I]
    return buf


def _unpack_z(buf, B, L):
    I = L // 2
    z = buf.reshape(2, B, 16, I).transpose(1, 2, 0, 3).reshape(B, 16, L).copy()
    z[:, :, :IGST] = z[:, :, IGST:IGST + 1]
    z[:, :, L - IGST:] = z[:, :, L - IGST - 1:L - IGST]
    return z


def _build_nc(n_steps, I, TW, nfuse):
    assert I % TW == 0 and n_steps % nfuse == 0
    NT = I // TW
    R = RADN
    C = I + 2 * R
    n = nfuse
    Wz = 12 * n + 24
    WA = Wz - 4
    napp = n_steps // n
    repl = 6 * n
    nc = bacc.Bacc(None, target_bir_lowering=False)

    zin = nc.declare_dram_parameter("zin", [128, C], F32, isOutput=False)
    win = nc.declare_dram_parameter("wbd", [NKT * 128, 128], F32R, isOutput=False)
    wmb_in = nc.declare_dram_parameter("wmb", [10 * 128, 128], BF16, isOutput=False)
    zout_d = nc.declare_dram_parameter("zout", [128, I], F32, isOutput=True)

    with TileContext(nc) as tc:
        with tc.tile_pool(name="state", bufs=1) as st, \
             tc.tile_pool(name="wpool", bufs=1) as wp, \
             tc.tile_pool(name="psA", bufs=5, space="PSUM") as psA, \
             tc.tile_pool(name="psS", bufs=3, space="PSUM") as psS:
            za = st.tile([128, C], F32, tag="zA", name="za")
            zb = st.tile([128, C], F32, tag="zB", name="zb")
            zbufs = [za, zb]
            zr = st.tile([128, C], F32R, tag="zr", name="zr")
            zsA = st.tile([128, Wz], F32, tag="zsA", name="zsA")
            zsB = st.tile([128, Wz], F32, tag="zsB", name="zsB")
            zsbufs = [zsA, zsB]
            zsr = st.tile([128, Wz], BF16, tag="zsr", name="zsr")
            k1s = st.tile([128, Wz], BF16, tag="k1s", name="k1s")
            k2s = st.tile([128, Wz], BF16, tag="k2s", name="k2s")
            wf, wm = [], {}
            for j in range(NKT):
                w = wp.tile([128, 128], F32R, tag=f"wf{j}", name=f"wf{j}")
                nc.sync.dma_start(out=w[:, :], in_=win[j * 128:(j + 1) * 128, :])
                wf.append(w)
            for v in range(2):
                for k in range(5):
                    idx = v * 5 + k
                    w = wp.tile([128, 128], BF16, tag=f"wm{idx}", name=f"wm{idx}")
                    nc.sync.dma_start(out=w[:, :],
                                      in_=wmb_in[idx * 128:(idx + 1) * 128, :])
                    wm[(v, k)] = w

            nc.vector.memset(k1s[:, :], 0.0)
            nc.vector.memset(k2s[:, :], 0.0)
            nc.sync.dma_start(out=za[:, :], in_=zin[:, :])
            for t in range(NT):
                s = R + TW * t
                nc.scalar.copy(zr[:, s:s + TW], za[:, s:s + TW])
            nc.scalar.copy(zr[:, 0:R], za[:, 0:R])
            nc.scalar.copy(zr[:, R + I:C], za[:, R + I:C])

            for app in range(napp):
                z = zbufs[app % 2]
                znew = zbufs[(app + 1) % 2]
                last = app == napp - 1

                nc.scalar.copy(zsA[0:64, :], z[0:64, R + 8:R + 8 + Wz])
                nc.scalar.copy(zsA[64:128, :], z[64:128, I + R - 8 - Wz:I + R - 8])
                nc.vector.tensor_copy(zsr[0:64, :], z[0:64, R + 8:R + 8 + Wz])
                nc.vector.tensor_copy(zsr[64:128, :], z[64:128, I + R - 8 - Wz:I + R - 8])

                def main_tile(t):
                    a = psA.tile([128, TW], F32, tag="A", name=f"A{app}_{t}")
                    for j in range(NKT):
                        nc.tensor.matmul(a[:, :], wf[j][:, :],
                                         zr[:, TW * t + j: TW * t + j + TW],
                                         start=(j == 0), stop=(j == NKT - 1),
                                         skip_group_check=True)
                    s = R + TW * t
                    nc.vector.tensor_tensor(znew[:, s:s + TW], a[:, :], z[:, s:s + TW],
                                            ALU.add)

                main_at = {}
                for idx in range(NT):
                    main_at.setdefault(min(2 * idx, n - 1), []).append(idx)

                for k_step in range(n):
                    zs = zsbufs[k_step % 2]
                    zs_n = zsbufs[(k_step + 1) % 2]
                    As = psS.tile([128, WA], F32, tag="As", name=f"As{app}_{k_step}")

                    def sstage(v, rhs, first=False):
                        for k in range(5):
                            nc.tensor.matmul(As[:, :], wm[(v, k)][:, :],
                                             rhs[:, k:k + WA],
                                             start=(first and k == 0),
                                             stop=(v == 1 and k == 4),
                                             skip_group_check=True)

                    def sfix(buf):
                        nc.vector.tensor_copy(
                            buf[0:64, 0:2], buf[0:64, 2:3].broadcast_to([64, 2]))
                        nc.vector.tensor_copy(
                            buf[64:128, Wz - 2:Wz],
                            buf[64:128, Wz - 3:Wz - 2].broadcast_to([64, 2]))

                    sstage(0, zsr, first=True)
                    nc.vector.tensor_tensor(k1s[:, 2:2 + WA], As[:, :],
                                            zs[:, 2:2 + WA], ALU.add)
                    sfix(k1s)
                    sstage(0, k1s)
                    nc.vector.scalar_tensor_tensor(k2s[:, 2:2 + WA], As[:, :], 0.25,
                                                   zs[:, 2:2 + WA], ALU.mult, ALU.add)
                    sfix(k2s)
                    sstage(1, k2s)
                    nc.vector.scalar_tensor_tensor(zs_n[:, 2:2 + WA], As[:, :],
                                                   1.0 / 6.0, zs[:, 2:2 + WA],
                                                   ALU.mult, ALU.add)
                    sfix(zs_n)
                    if k_step < n - 1:
                        nc.vector.tensor_copy(zsr[:, :], zs_n[:, :])
                    for t in main_at.get(k_step, []):
                        main_tile(t)

                zs_fin = zsbufs[n % 2]
                nc.vector.tensor_copy(znew[0:64, R + 10:R + 10 + repl],
                                      zs_fin[0:64, 2:2 + repl])
                nc.vector.tensor_copy(znew[64:128, I - repl:I],
                                      zs_fin[64:128, Wz - 2 - repl:Wz - 2])

                if not last:
                    nc.scalar.copy(znew[0:64, R + 8:R + 10],
                                   znew[0:64, R + 10:R + 11].broadcast_to([64, 2]))
                    nc.scalar.copy(znew[64:128, I:I + 2],
                                   znew[64:128, I - 1:I].broadcast_to([64, 2]))
                    nc.sync.dma_start(out=znew[0:64, R + I:C], in_=znew[64:128, R:2 * R])
                    nc.sync.dma_start(out=znew[64:128, 0:R], in_=znew[0:64, I:I + R])
                    for t in range(NT):
                        s = R + TW * t
                        nc.scalar.copy(zr[:, s:s + TW], znew[:, s:s + TW])
                    nc.scalar.copy(zr[:, 0:R], znew[:, 0:R])
                    nc.scalar.copy(zr[:, R + I:C], znew[:, R + I:C])

            zfin = zbufs[napp % 2]
            nc.sync.dma_start(out=zout_d[:, :], in_=zfin[:, R:R + I])

    nc.finalize()
    return nc


_NC_CACHE = {}


def _pick_nfuse(n_steps):
    for n in (20, 10, 5, 4, 2, 1):
        if n_steps % n == 0 and 6 * n < 1500:
            return n
    return 1


def kernel(z0, W, t1_t0):
    z0 = np.asarray(z0, dtype=np.float32)
    W = np.asarray(W, dtype=np.float32)
    t = int(np.asarray(t1_t0))
    if t == 0:
        return z0.copy()
    n_steps = int(round(t / EPS))
    h = t / n_steps
    B, nch, L = z0.shape
    I = L // 2
    BPC = B // NCORES
    nfuse = _pick_nfuse(n_steps)

    key = (n_steps, I, nfuse)
    if key not in _NC_CACHE:
        _NC_CACHE[key] = _build_nc(n_steps, I, 512, nfuse)
    nc = _NC_CACHE[key]

    wbd, wmb = _make_weights(W, h, nfuse)
    in_maps = [{"zin": _pack_z(z0[c * BPC:(c + 1) * BPC]), "wbd": wbd, "wmb": wmb}
               for c in range(NCORES)]
    br = run_bass_kernel_spmd(nc, in_maps, list(range(NCORES)))
    out = np.concatenate(
        [_unpack_z(br.results[c]["zout"], BPC, L) for c in range(NCORES)], axis=0)
    return out.astype(np.float32)
